# revision 37
# baseline (speedup 1.0000x reference)
"""Trainium2 Bass kernel for the gnn_message_passing encoder problem.

kernel(**inputs) takes the FULL inputs and returns the FULL [B, P, R+1] output.

Sharding: 8 cores = 2 batches x 4 object-groups; each core scores 64 padded
(trigger, object) pairs of one document.  Host ships only the gathered
attention rows (bf16, [head-pair, (e,w), L] tiles), the full batch
sequence_output in L-chunk-major layout with a fused ones-column (so the
q row-sum rides the context matmul for free), span token rows, selector
matrices, and the transposed codebooks.

Device pipeline per core:
  1. Pair expansion on PE with block-diagonal one-hot selectors (W-sum is
     folded into the selectors; its scale cancels in the q/qsum ratio):
     two [128,512] matmuls per (head-pair, L-quarter) produce a_s / a_o
     in PSUM at M=128.
  2. a_s staged PSUM->SBUF bf16 on the scalar engine; products
     pm = a_s * a_o on vector (2/3) and gpsimd (1/3) engines.
  3. Head-sum tree on vector (scalar_tensor_tensor, all-SBUF bf16),
     leaving two head-half copies per partition-half (no fold yet).
  4. PE transposes q2 [128,128] chunks into bf16 PSUM (l on partitions),
     scalar engine copies them to SBUF; context matmuls accumulate
     c2 = q2T^T @ [seq | 1] over all 16 L-chunks (col 768 = qsum).
  5. Tail: fold the two head-halves of c2 with a [I;I] matmul, reciprocal
     of qsum, normalize c on the scalar engine (per-partition scale AP),
     transpose into the f-major embs layout, one 18-chunk scoring matmul
     against [rel; nota], transpose, NOTA max, output DMA.
"""

import os
import sys

import numpy as np

for _p in ("/opt/trn_rl_repo", os.path.expanduser("~/.axon_site/_ro/trn_rl_repo")):
    if os.path.isdir(_p) and _p not in sys.path:
        sys.path.insert(0, _p)

import concourse.bass as bass
import concourse.mybir as mybir
import concourse.tile as tile
from concourse import bacc
from concourse.bass_utils import run_bass_kernel_spmd

# Problem dimensions (hardcoded per the harness contract).
B, L, D, H = 2, 2048, 768, 12
E, T, W = 32, 8, 4
R, NN = 57, 20
RN = R + NN            # 77 stacked codebook rows
NE = 16                # entities per core (8 triggers + 8 objects)
NEW = NE * W           # 64 gathered rows per head
NP = 64                # pair slots per core (group 0 pads 56 -> 64)
LQ = 512               # L is processed in 4 slices of 512
DSEQ = D + 8           # seq free dim with ones column at 768 (pad to 776)
NCORES = 8

# Static pair list in the reference's order (s-major).
ALL_PAIRS = [(s, o) for s in range(T) for o in range(E) if s != o]
GROUP_IDX = [[i for i, (_, o) in enumerate(ALL_PAIRS) if o // 8 == g] for g in range(4)]
GROUP_ENTS = [
    list(range(16)),
    list(range(16)),
    list(range(8)) + list(range(16, 24)),
    list(range(8)) + list(range(24, 32)),
]

F32 = mybir.dt.float32
BF16 = mybir.dt.bfloat16
FP8 = mybir.dt.float8e4
import ml_dtypes
NP_BF16 = ml_dtypes.bfloat16
NP_FP8 = ml_dtypes.float8_e4m3

LAST_RESULTS = None  # BassKernelResults of the most recent kernel() call


def _sel_matrices(g):
    """Attention selectors (1.0; scale-free) and entity selectors (0.25)."""
    idxs = GROUP_IDX[g]
    ents = GROUP_ENTS[g]
    local = {e: i for i, e in enumerate(ents)}
    sel_s = np.zeros((NEW, NP), np.float32)
    sel_o = np.zeros((NEW, NP), np.float32)
    for j in range(NP):
        s, o = ALL_PAIRS[idxs[j % len(idxs)]]  # pad group 0 by repeating pair 0
        for w in range(W):
            sel_s[local[s] * W + w, j] = 1.0
            sel_o[local[o] * W + w, j] = 1.0
    return sel_s, sel_o


def _sel_doublerow(g):
    """DoubleRow expansion selectors [side, t, 128, 2, 128].

    k-tile partition p = h_loc*32 + e*2 + wg holds att rows (head 4*st +
    h_loc, entity e, w = 2*wg + j) in slot j.  Output partition m = hh*64 +
    pair covers heads (2t+hh) of the supertile; the W-sum spreads 1.0 over
    all four (wg, j) combinations (its scale cancels in q/qsum).
    """
    idxs = GROUP_IDX[g]
    ents = GROUP_ENTS[g]
    local = {e: i for i, e in enumerate(ents)}
    dr = np.zeros((2, 2, 128, 2, 128), np.float32)
    for j in range(NP):
        s, o = ALL_PAIRS[idxs[j % len(idxs)]]
        for side, ent in ((0, s), (1, o)):
            el = local[ent]
            for t in range(2):
                for hh in range(2):
                    m = hh * NP + j
                    h_loc = 2 * t + hh
                    for wg in range(2):
                        for js in range(2):
                            dr[side, t, h_loc * 32 + el * 2 + wg, js, m] = 1.0
    return dr


def _build_program(debug=False):
    nc = bacc.Bacc("TRN2")

    att_g = nc.dram_tensor("att_g", [4, 128, 6 * LQ], FP8, kind="ExternalInput")
    seq = nc.dram_tensor("seq", [128, 16 * DSEQ], FP8, kind="ExternalInput")
    spans = nc.dram_tensor("spans", [NEW, D], BF16, kind="ExternalInput")
    selb = nc.dram_tensor("selb", [128, 2 * 128], FP8, kind="ExternalInput")
    sele = nc.dram_tensor("sele", [NEW, 2 * NP], BF16, kind="ExternalInput")
    rel_t = nc.dram_tensor("rel_t", [128, 18 * RN], BF16, kind="ExternalInput")
    out = nc.dram_tensor("out", [NP, R + 1], F32, kind="ExternalOutput")
    if debug:
        dbg_q = nc.dram_tensor("dbg_q", [128, 16 * 128], F32, kind="ExternalOutput")
        dbg_c2 = nc.dram_tensor("dbg_c2", [128, DSEQ], F32, kind="ExternalOutput")
        dbg_emb = nc.dram_tensor("dbg_emb", [128, 18 * NP], F32, kind="ExternalOutput")

    mult = mybir.AluOpType.mult
    addop = mybir.AluOpType.add

    with tile.TileContext(nc) as tc:
        with tc.tile_pool(name="consts", bufs=1) as consts:
            # Small inputs first so the entity phase and expansion can start
            # immediately; seq is split in halves and interleaved between
            # attention quarters so the first context matmuls aren't starved.
            selb_sb = consts.tile([128, 2, 128], FP8)
            nc.sync.dma_start(out=selb_sb, in_=selb.rearrange("p (s n) -> p s n", s=2))
            sele_sb = consts.tile([NEW, 2, NP], BF16)
            nc.sync.dma_start(out=sele_sb, in_=sele.rearrange("p (s n) -> p s n", s=2))
            spans_sb = consts.tile([NEW, D], BF16)
            nc.sync.dma_start(out=spans_sb, in_=spans[:, :])
            rel_sb = consts.tile([128, 18, RN], BF16)
            nc.sync.dma_start(out=rel_sb, in_=rel_t.rearrange("p (c n) -> p c n", c=18))
            g_sb = consts.tile([128, 6, L], FP8)
            g_view = att_g.rearrange("q p (t l) -> q p t l", t=6)
            nc.sync.dma_start(out=g_sb[:, :, 0:LQ], in_=g_view[0])
            nc.sync.dma_start(out=g_sb[:, :, LQ:2 * LQ], in_=g_view[1])
            seq_sb = consts.tile([128, 16, DSEQ], FP8)
            seq_view = seq.rearrange("p (c d) -> p c d", c=16)
            nc.sync.dma_start(out=seq_sb[:, 0:8, :], in_=seq_view[:, 0:8, :])
            nc.sync.dma_start(out=g_sb[:, :, 2 * LQ:3 * LQ], in_=g_view[2])
            nc.sync.dma_start(out=seq_sb[:, 8:16, :], in_=seq_view[:, 8:16, :])
            nc.sync.dma_start(out=g_sb[:, :, 3 * LQ:4 * LQ], in_=g_view[3])

            # fold2 = [I64; I64] and a bf16 identity, built on device.
            fold2_sb = consts.tile([128, NP], BF16)
            nc.gpsimd.memset(fold2_sb, 0.0)
            nc.gpsimd.affine_select(
                out=fold2_sb[0:NP, :], in_=fold2_sb[0:NP, :],
                compare_op=mybir.AluOpType.not_equal, fill=1.0, base=0,
                pattern=[[-1, NP]], channel_multiplier=1,
            )
            nc.gpsimd.affine_select(
                out=fold2_sb[NP:128, :], in_=fold2_sb[NP:128, :],
                compare_op=mybir.AluOpType.not_equal, fill=1.0, base=0,
                pattern=[[-1, NP]], channel_multiplier=1,
            )
            idb_sb = consts.tile([128, 128], BF16)
            nc.gpsimd.memset(idb_sb, 0.0)
            nc.gpsimd.affine_select(
                out=idb_sb, in_=idb_sb,
                compare_op=mybir.AluOpType.not_equal, fill=1.0, base=0,
                pattern=[[-1, 128]], channel_multiplier=1,
            )

            qT2sb = consts.tile([128, 16, 128], FP8)
            embsT = consts.tile([128, 18, NP], BF16)
            c2sb = consts.tile([128, DSEQ], BF16)
            csb = consts.tile([NP, D], BF16)
            scsb = consts.tile([RN, NP], BF16)
            rq = consts.tile([NP, 1], F32)
            nota_t = consts.tile([NP, 1], F32)
            fin = consts.tile([NP, R + 1], F32)

            # Entity phase first: it only needs spans+sele+rel, so it runs
            # while the attention quarters are still streaming in.  The
            # entity part of the scores (rel chunks 0..11) is also fully
            # computed here; the c-part joins at the tail.  Its PSUM pool
            # closes before the main pipeline's pools open.
            sceT = consts.tile([NP, RN], BF16)
            with tc.tile_pool(name="psE", bufs=1, space="PSUM") as psE:
                e_ps0 = psE.tile([128, 3, 128], F32, tag="e0")
                e_ps1 = psE.tile([128, 3, 128], F32, tag="e1")
                for db in range(6):
                    e_ps = e_ps0 if db < 3 else e_ps1
                    nc.tensor.matmul(
                        out=e_ps[:, db % 3, :],
                        lhsT=spans_sb[:, db * 128:(db + 1) * 128],
                        rhs=sele_sb.rearrange("p s n -> p (s n)"))
                nc.scalar.copy(embsT[:, 0:3, :], e_ps0[:, :, 0:NP])
                nc.vector.tensor_copy(embsT[:, 6:9, :], e_ps0[:, :, NP:128])
                nc.scalar.copy(embsT[:, 3:6, :], e_ps1[:, :, 0:NP])
                nc.vector.tensor_copy(embsT[:, 9:12, :], e_ps1[:, :, NP:128])
                sce_ps = psE.tile([RN, NP], F32, tag="sce")
                for kc in range(12):
                    nc.tensor.matmul(
                        out=sce_ps, lhsT=rel_sb[:, kc, :], rhs=embsT[:, kc, :],
                        start=(kc == 0), stop=(kc == 11))
                scesb = consts.tile([RN, NP], BF16)
                nc.scalar.copy(scesb, sce_ps)
                sceT_ps = psE.tile([NP, RN], BF16, tag="sceT")
                nc.tensor.matmul(out=sceT_ps, lhsT=scesb, rhs=idb_sb[0:RN, 0:RN],
                                 is_transpose=True)
                nc.vector.tensor_copy(sceT, sceT_ps)

            # Main pipeline: PSUM = psA(2+3=5) + psT(2x0.5=1) + psC(2) = 8 banks.
            with tc.tile_pool(name="psT", bufs=1, space="PSUM") as psT, \
                 tc.tile_pool(name="psC", bufs=1, space="PSUM") as psC:
                c2a = psC.tile([128, 384], F32, tag="c2a")
                c2b = psC.tile([128, DSEQ - 384], F32, tag="c2b")

                with tc.tile_pool(name="psA", bufs=2, space="PSUM") as psA, \
                     tc.tile_pool(name="prod", bufs=2) as prod, \
                     tc.tile_pool(name="stg", bufs=3) as stg, \
                     tc.tile_pool(name="q2p", bufs=2) as q2p:
                    for lq in range(4):
                        pmt = prod.tile([128, 6, LQ], BF16, tag="pm")
                        for hp in range(6):
                            u = lq * 6 + hp
                            a_s = psA.tile([128, LQ], F32, tag="as", bufs=2)
                            a_o = psA.tile([128, LQ], F32, tag="ao", bufs=3)
                            rhs = g_sb[:, hp, lq * LQ:(lq + 1) * LQ]
                            nc.tensor.matmul(out=a_s, lhsT=selb_sb[:, 0, :], rhs=rhs)
                            nc.tensor.matmul(out=a_o, lhsT=selb_sb[:, 1, :], rhs=rhs)
                            # DVE may read at most one PSUM operand: a_o
                            # stays in PSUM, a_s comes via a staged copy.
                            # (gpsimd cannot access PSUM at all.)
                            ss = stg.tile([128, LQ], BF16, tag="ss")
                            nc.scalar.copy(ss, a_s)
                            nc.vector.tensor_mul(pmt[:, hp, :], ss, a_o)
                        # head-sum tree: pairwise plain adds (2x bf16 mode)
                        nc.gpsimd.tensor_add(pmt[:, 0, :], pmt[:, 0, :], pmt[:, 1, :])
                        nc.gpsimd.tensor_add(pmt[:, 2, :], pmt[:, 2, :], pmt[:, 3, :])
                        nc.gpsimd.tensor_add(pmt[:, 4, :], pmt[:, 4, :], pmt[:, 5, :])
                        q2t = q2p.tile([128, LQ], BF16, tag="q2")
                        nc.vector.tensor_add(pmt[:, 0, :], pmt[:, 0, :], pmt[:, 2, :])
                        nc.vector.tensor_add(q2t, pmt[:, 0, :], pmt[:, 4, :])
                        # transpose q2 into l-major bf16 PSUM
                        qt_ps = psT.tile([128, 4, 128], BF16, tag="qt", bufs=1)
                        for k in range(4):
                            nc.tensor.matmul(
                                out=qt_ps[:, k, :],
                                lhsT=q2t[:, k * 128:(k + 1) * 128],
                                rhs=idb_sb, is_transpose=True)
                        # stage this lq's qT2 chunks to SBUF and run their
                        # context matmuls right away (spreads ctx + tail work)
                        if lq % 2 == 0:
                            nc.scalar.copy(qT2sb[:, lq * 4:lq * 4 + 4, :], qt_ps)
                        else:
                            nc.vector.tensor_copy(qT2sb[:, lq * 4:lq * 4 + 4, :], qt_ps)
                        for cp in (lq * 4, lq * 4 + 2):
                            nc.tensor.matmul(
                                out=c2a, lhsT=qT2sb[:, cp:cp + 2, :],
                                rhs=seq_sb[:, cp:cp + 2, 0:384],
                                start=(cp == 0), stop=(cp == 14),
                                perf_mode=mybir.MatmulPerfMode.DoubleRow)
                            nc.tensor.matmul(
                                out=c2b, lhsT=qT2sb[:, cp:cp + 2, :],
                                rhs=seq_sb[:, cp:cp + 2, 384:DSEQ],
                                start=(cp == 0), stop=(cp == 14),
                                perf_mode=mybir.MatmulPerfMode.DoubleRow)
                    # stage c2 out of PSUM (frees psC banks for the tail)
                    nc.scalar.copy(c2sb[:, 0:384], c2a)
                    nc.vector.tensor_copy(c2sb[:, 384:DSEQ], c2b)

            # Tail: fold, unnormalized c scores, then combine with the
            # entity scores scaled by 1/qsum.  <= 6 PSUM banks.
            with tc.tile_pool(name="psF", bufs=1, space="PSUM") as psF:
                # fold head-halves: c2 -> c (col 768 = qsum); c stays
                # unnormalized, the 1/qsum lands on the c-scores at the end.
                c_psa = psF.tile([NP, 512], F32, tag="ca")
                c_psb = psF.tile([NP, DSEQ - 512], F32, tag="cb")
                nc.tensor.matmul(out=c_psa, lhsT=fold2_sb, rhs=c2sb[:, 0:512])
                nc.tensor.matmul(out=c_psb, lhsT=fold2_sb, rhs=c2sb[:, 512:DSEQ])
                nc.scalar.copy(csb[:, 0:512], c_psa)
                nc.vector.tensor_copy(csb[:, 512:D], c_psb[:, 0:D - 512])
                nc.vector.reciprocal(rq, c_psb[:, D - 512:D - 512 + 1])
                cT_ps = psF.tile([128, 6, NP], BF16, tag="cT")
                for db in range(6):
                    nc.tensor.matmul(
                        out=cT_ps[:, db, :],
                        lhsT=csb[:, db * 128:(db + 1) * 128],
                        rhs=idb_sb[0:NP, 0:NP], is_transpose=True)
                nc.scalar.copy(embsT[:, 12:18, :], cT_ps)

                # c-part scores only (rel chunks 12..17), transpose, combine
                sc_ps = psF.tile([RN, NP], F32, tag="sc")
                for kc in range(12, 18):
                    nc.tensor.matmul(
                        out=sc_ps, lhsT=rel_sb[:, kc, :], rhs=embsT[:, kc, :],
                        start=(kc == 12), stop=(kc == 17))
                nc.scalar.copy(scsb, sc_ps)
                scT_ps = psF.tile([NP, RN], BF16, tag="scT")
                nc.tensor.matmul(out=scT_ps, lhsT=scsb, rhs=idb_sb[0:RN, 0:RN],
                                 is_transpose=True)
                ft = consts.tile([NP, RN], F32)
                nc.vector.scalar_tensor_tensor(
                    out=ft, in0=scT_ps, scalar=rq, in1=sceT,
                    op0=mult, op1=addop)
                nc.vector.reduce_max(nota_t, ft[:, R:RN],
                                     axis=mybir.AxisListType.X)
                nc.vector.tensor_copy(fin[:, 1:R + 1], ft[:, 0:R])
                nc.vector.tensor_copy(fin[:, 0:1], nota_t)

            nc.sync.dma_start(out=out[:, :], in_=fin)
            if debug:
                dq = consts.tile([128, 16, 128], F32)
                nc.vector.tensor_copy(dq, qT2sb)
                nc.sync.dma_start(out=dbg_q.rearrange("p (c n) -> p c n", c=16), in_=dq)
                dc2 = consts.tile([128, DSEQ], F32)
                nc.vector.tensor_copy(dc2, c2sb)
                nc.sync.dma_start(out=dbg_c2[:, :], in_=dc2)
                de = consts.tile([128, 18, NP], F32)
                nc.vector.tensor_copy(de, embsT)
                nc.sync.dma_start(out=dbg_emb.rearrange("p (c n) -> p c n", c=18), in_=de)

    return nc


def _in_maps(sequence_output, attention, relation_embeddings, nota_embeddings,
             span_starts):
    sequence_output = np.asarray(sequence_output, np.float32)
    attention = np.asarray(attention, np.float32)
    span_starts = np.asarray(span_starts)
    rel_t = np.ascontiguousarray(
        np.concatenate(
            [np.asarray(relation_embeddings, np.float32),
             np.asarray(nota_embeddings, np.float32)], axis=0
        ).T
    )
    rel_pm = rel_t.astype(NP_BF16).reshape(18, 128, RN).transpose(1, 0, 2)

    in_maps = []
    for c in range(NCORES):
        b, g = divmod(c, 4)
        ents = GROUP_ENTS[g]
        rows = np.concatenate(
            [np.arange(span_starts[b, e], span_starts[b, e] + W) for e in ents]
        )
        att_rows = attention[b][:, rows, :].reshape(H * NEW, L)
        sel_s, sel_o = _sel_matrices(g)
        att_q = att_rows.astype(NP_FP8).reshape(6, 128, 4, LQ).transpose(2, 1, 0, 3)
        seq_pm = np.zeros((128, 16, DSEQ), NP_FP8)
        seq_pm[:, :, 0:D] = sequence_output[b].astype(NP_FP8).reshape(16, 128, D).transpose(1, 0, 2)
        seq_pm[:, :, D] = 1.0
        def _bd(m):
            z = np.zeros_like(m)
            return np.block([[m, z], [z, m]])
        selb_h = np.concatenate([_bd(sel_s), _bd(sel_o)], axis=1)  # [128, 256]
        sele_h = np.concatenate([sel_s * 0.25, sel_o * 0.25], axis=1)  # [64, 128]
        in_maps.append({
            "att_g": np.ascontiguousarray(att_q.reshape(4, 128, 6 * LQ)),
            "seq": np.ascontiguousarray(seq_pm.reshape(128, 16 * DSEQ)),
            "spans": np.ascontiguousarray(sequence_output[b][rows].astype(NP_BF16)),
            "selb": np.ascontiguousarray(selb_h.astype(NP_FP8)),
            "sele": np.ascontiguousarray(sele_h.astype(NP_BF16)),
            "rel_t": np.ascontiguousarray(rel_pm.reshape(128, 18 * RN)),
        })
    return in_maps


def kernel(sequence_output, attention, relation_embeddings, nota_embeddings,
           span_starts):
    global LAST_RESULTS
    in_maps = _in_maps(sequence_output, attention, relation_embeddings,
                       nota_embeddings, span_starts)
    nc = _build_program()
    nc.finalize()  # Bacc legalization (wait splitting, reg alloc)
    LAST_RESULTS = run_bass_kernel_spmd(nc, in_maps, core_ids=list(range(NCORES)))

    out = np.zeros((B, len(ALL_PAIRS), R + 1), np.float32)
    for c in range(NCORES):
        b, g = divmod(c, 4)
        idxs = GROUP_IDX[g]
        out[b, idxs, :] = LAST_RESULTS.results[c]["out"][: len(idxs)]
    return out


# revision 38
# speedup vs baseline: 1.1516x; 1.1516x over previous
"""Trainium2 Bass kernel for the gnn_message_passing encoder problem.

kernel(**inputs) takes the FULL inputs and returns the FULL [B, P, R+1] output.

Sharding: 8 cores = 2 batches x 4 object-groups; each core scores 64 padded
(trigger, object) pairs of one document.  Host ships only the gathered
attention rows (bf16, [head-pair, (e,w), L] tiles), the full batch
sequence_output in L-chunk-major layout with a fused ones-column (so the
q row-sum rides the context matmul for free), span token rows, selector
matrices, and the transposed codebooks.

Device pipeline per core:
  1. Pair expansion on PE with block-diagonal one-hot selectors (W-sum is
     folded into the selectors; its scale cancels in the q/qsum ratio):
     two [128,512] matmuls per (head-pair, L-quarter) produce a_s / a_o
     in PSUM at M=128.
  2. a_s staged PSUM->SBUF bf16 on the scalar engine; products
     pm = a_s * a_o on vector (2/3) and gpsimd (1/3) engines.
  3. Head-sum tree on vector (scalar_tensor_tensor, all-SBUF bf16),
     leaving two head-half copies per partition-half (no fold yet).
  4. PE transposes q2 [128,128] chunks into bf16 PSUM (l on partitions),
     scalar engine copies them to SBUF; context matmuls accumulate
     c2 = q2T^T @ [seq | 1] over all 16 L-chunks (col 768 = qsum).
  5. Tail: fold the two head-halves of c2 with a [I;I] matmul, reciprocal
     of qsum, normalize c on the scalar engine (per-partition scale AP),
     transpose into the f-major embs layout, one 18-chunk scoring matmul
     against [rel; nota], transpose, NOTA max, output DMA.
"""

import os
import sys

import numpy as np

for _p in ("/opt/trn_rl_repo", os.path.expanduser("~/.axon_site/_ro/trn_rl_repo")):
    if os.path.isdir(_p) and _p not in sys.path:
        sys.path.insert(0, _p)

import concourse.bass as bass
import concourse.mybir as mybir
import concourse.tile as tile
from concourse import bacc
from concourse.bass_utils import run_bass_kernel_spmd

# Problem dimensions (hardcoded per the harness contract).
B, L, D, H = 2, 2048, 768, 12
E, T, W = 32, 8, 4
R, NN = 57, 20
RN = R + NN            # 77 stacked codebook rows
NE = 16                # entities per core (8 triggers + 8 objects)
NEW = NE * W           # 64 gathered rows per head
NP = 64                # pair slots per core (group 0 pads 56 -> 64)
LQ = 512               # L is processed in 4 slices of 512
DSEQ = D + 8           # seq free dim with ones column at 768 (pad to 776)
NCORES = 8

# Static pair list in the reference's order (s-major).
ALL_PAIRS = [(s, o) for s in range(T) for o in range(E) if s != o]
GROUP_IDX = [[i for i, (_, o) in enumerate(ALL_PAIRS) if o // 8 == g] for g in range(4)]
GROUP_ENTS = [
    list(range(16)),
    list(range(16)),
    list(range(8)) + list(range(16, 24)),
    list(range(8)) + list(range(24, 32)),
]

F32 = mybir.dt.float32
BF16 = mybir.dt.bfloat16
FP8 = mybir.dt.float8e4
import ml_dtypes
NP_BF16 = ml_dtypes.bfloat16
NP_FP8 = ml_dtypes.float8_e4m3

LAST_RESULTS = None  # BassKernelResults of the most recent kernel() call


def _sel_matrices(g):
    """Attention selectors (1.0; scale-free) and entity selectors (0.25)."""
    idxs = GROUP_IDX[g]
    ents = GROUP_ENTS[g]
    local = {e: i for i, e in enumerate(ents)}
    sel_s = np.zeros((NEW, NP), np.float32)
    sel_o = np.zeros((NEW, NP), np.float32)
    for j in range(NP):
        s, o = ALL_PAIRS[idxs[j % len(idxs)]]  # pad group 0 by repeating pair 0
        for w in range(W):
            sel_s[local[s] * W + w, j] = 1.0
            sel_o[local[o] * W + w, j] = 1.0
    return sel_s, sel_o


def _sel_doublerow(g):
    """DoubleRow expansion selectors [side, t, 128, 2, 128].

    k-tile partition p = h_loc*32 + e*2 + wg holds att rows (head 4*st +
    h_loc, entity e, w = 2*wg + j) in slot j.  Output partition m = hh*64 +
    pair covers heads (2t+hh) of the supertile; the W-sum spreads 1.0 over
    all four (wg, j) combinations (its scale cancels in q/qsum).
    """
    idxs = GROUP_IDX[g]
    ents = GROUP_ENTS[g]
    local = {e: i for i, e in enumerate(ents)}
    dr = np.zeros((2, 2, 128, 2, 128), np.float32)
    for j in range(NP):
        s, o = ALL_PAIRS[idxs[j % len(idxs)]]
        for side, ent in ((0, s), (1, o)):
            el = local[ent]
            for t in range(2):
                for hh in range(2):
                    m = hh * NP + j
                    h_loc = 2 * t + hh
                    for wg in range(2):
                        for js in range(2):
                            dr[side, t, h_loc * 32 + el * 2 + wg, js, m] = 1.0
    return dr


def _build_program(debug=False):
    nc = bacc.Bacc("TRN2")

    att_g = nc.dram_tensor("att_g", [4, 128, 6 * LQ], FP8, kind="ExternalInput")
    seq = nc.dram_tensor("seq", [128, 16 * DSEQ], FP8, kind="ExternalInput")
    spans = nc.dram_tensor("spans", [NEW, D], BF16, kind="ExternalInput")
    selb = nc.dram_tensor("selb", [128, 2 * 128], FP8, kind="ExternalInput")
    sele = nc.dram_tensor("sele", [NEW, 2 * NP], BF16, kind="ExternalInput")
    rel_t = nc.dram_tensor("rel_t", [128, 18 * RN], BF16, kind="ExternalInput")
    out = nc.dram_tensor("out", [NP, R + 1], F32, kind="ExternalOutput")
    if debug:
        dbg_q = nc.dram_tensor("dbg_q", [128, 16 * 128], F32, kind="ExternalOutput")
        dbg_c2 = nc.dram_tensor("dbg_c2", [128, DSEQ], F32, kind="ExternalOutput")
        dbg_emb = nc.dram_tensor("dbg_emb", [128, 18 * NP], F32, kind="ExternalOutput")

    mult = mybir.AluOpType.mult
    addop = mybir.AluOpType.add

    with tile.TileContext(nc) as tc:
        with tc.tile_pool(name="consts", bufs=1) as consts:
            # Small inputs first so the entity phase and expansion can start
            # immediately; seq is split in halves and interleaved between
            # attention quarters so the first context matmuls aren't starved.
            selb_sb = consts.tile([128, 2, 128], FP8)
            nc.sync.dma_start(out=selb_sb, in_=selb.rearrange("p (s n) -> p s n", s=2))
            sele_sb = consts.tile([NEW, 2, NP], BF16)
            nc.sync.dma_start(out=sele_sb, in_=sele.rearrange("p (s n) -> p s n", s=2))
            spans_sb = consts.tile([NEW, D], BF16)
            nc.sync.dma_start(out=spans_sb, in_=spans[:, :])
            rel_sb = consts.tile([128, 18, RN], BF16)
            nc.sync.dma_start(out=rel_sb, in_=rel_t.rearrange("p (c n) -> p c n", c=18))
            g_sb = consts.tile([128, 6, L], FP8)
            g_view = att_g.rearrange("q p (t l) -> q p t l", t=6)
            nc.sync.dma_start(out=g_sb[:, :, 0:LQ], in_=g_view[0])
            nc.sync.dma_start(out=g_sb[:, :, LQ:2 * LQ], in_=g_view[1])
            seq_sb = consts.tile([128, 16, DSEQ], FP8)
            seq_view = seq.rearrange("p (c d) -> p c d", c=16)
            nc.sync.dma_start(out=seq_sb[:, 0:8, :], in_=seq_view[:, 0:8, :])
            nc.sync.dma_start(out=g_sb[:, :, 2 * LQ:3 * LQ], in_=g_view[2])
            nc.sync.dma_start(out=seq_sb[:, 8:16, :], in_=seq_view[:, 8:16, :])
            nc.sync.dma_start(out=g_sb[:, :, 3 * LQ:4 * LQ], in_=g_view[3])

            # fold2 = [I64; I64] and a bf16 identity, built on device.
            fold2_sb = consts.tile([128, NP], BF16)
            nc.gpsimd.memset(fold2_sb, 0.0)
            nc.gpsimd.affine_select(
                out=fold2_sb[0:NP, :], in_=fold2_sb[0:NP, :],
                compare_op=mybir.AluOpType.not_equal, fill=1.0, base=0,
                pattern=[[-1, NP]], channel_multiplier=1,
            )
            nc.gpsimd.affine_select(
                out=fold2_sb[NP:128, :], in_=fold2_sb[NP:128, :],
                compare_op=mybir.AluOpType.not_equal, fill=1.0, base=0,
                pattern=[[-1, NP]], channel_multiplier=1,
            )
            idb_sb = consts.tile([128, 128], BF16)
            nc.gpsimd.memset(idb_sb, 0.0)
            nc.gpsimd.affine_select(
                out=idb_sb, in_=idb_sb,
                compare_op=mybir.AluOpType.not_equal, fill=1.0, base=0,
                pattern=[[-1, 128]], channel_multiplier=1,
            )

            qT2sb = consts.tile([128, 16, 128], FP8)
            embsT = consts.tile([128, 18, NP], BF16)
            c2sb = consts.tile([128, DSEQ], BF16)
            csb = consts.tile([NP, D], BF16)
            scsb = consts.tile([RN, NP], BF16)
            rq = consts.tile([NP, 1], F32)
            nota_t = consts.tile([NP, 1], F32)
            fin = consts.tile([NP, R + 1], F32)

            # Entity phase first: it only needs spans+sele+rel, so it runs
            # while the attention quarters are still streaming in.  The
            # entity part of the scores (rel chunks 0..11) is also fully
            # computed here; the c-part joins at the tail.  Its PSUM pool
            # closes before the main pipeline's pools open.
            sceT = consts.tile([NP, RN], BF16)
            with tc.tile_pool(name="psE", bufs=1, space="PSUM") as psE:
                e_ps0 = psE.tile([128, 3, 128], F32, tag="e0")
                e_ps1 = psE.tile([128, 3, 128], F32, tag="e1")
                for db in range(6):
                    e_ps = e_ps0 if db < 3 else e_ps1
                    nc.tensor.matmul(
                        out=e_ps[:, db % 3, :],
                        lhsT=spans_sb[:, db * 128:(db + 1) * 128],
                        rhs=sele_sb.rearrange("p s n -> p (s n)"))
                nc.scalar.copy(embsT[:, 0:3, :], e_ps0[:, :, 0:NP])
                nc.vector.tensor_copy(embsT[:, 6:9, :], e_ps0[:, :, NP:128])
                nc.scalar.copy(embsT[:, 3:6, :], e_ps1[:, :, 0:NP])
                nc.vector.tensor_copy(embsT[:, 9:12, :], e_ps1[:, :, NP:128])
                sce_ps = psE.tile([RN, NP], F32, tag="sce")
                for kc in range(12):
                    nc.tensor.matmul(
                        out=sce_ps, lhsT=rel_sb[:, kc, :], rhs=embsT[:, kc, :],
                        start=(kc == 0), stop=(kc == 11))
                scesb = consts.tile([RN, NP], BF16)
                nc.scalar.copy(scesb, sce_ps)
                sceT_ps = psE.tile([NP, RN], BF16, tag="sceT")
                nc.tensor.matmul(out=sceT_ps, lhsT=scesb, rhs=idb_sb[0:RN, 0:RN],
                                 is_transpose=True)
                nc.vector.tensor_copy(sceT, sceT_ps)

            # Main pipeline: PSUM = psA(2+3=5) + psT(2x0.5=1) + psC(2) = 8 banks.
            with tc.tile_pool(name="psT", bufs=1, space="PSUM") as psT, \
                 tc.tile_pool(name="psC", bufs=1, space="PSUM") as psC:
                c2a = psC.tile([128, 384], F32, tag="c2a")
                c2b = psC.tile([128, DSEQ - 384], F32, tag="c2b")

                with tc.tile_pool(name="psA", bufs=2, space="PSUM") as psA, \
                     tc.tile_pool(name="prod", bufs=2) as prod, \
                     tc.tile_pool(name="stg", bufs=3) as stg, \
                     tc.tile_pool(name="q2p", bufs=2) as q2p:
                    for lq in range(4):
                        pmt = prod.tile([128, 6, LQ], BF16, tag="pm")
                        for hp in range(6):
                            u = lq * 6 + hp
                            a_s = psA.tile([128, LQ], F32, tag="as", bufs=2)
                            a_o = psA.tile([128, LQ], F32, tag="ao", bufs=3)
                            rhs = g_sb[:, hp, lq * LQ:(lq + 1) * LQ]
                            nc.tensor.matmul(out=a_s, lhsT=selb_sb[:, 0, :], rhs=rhs)
                            nc.tensor.matmul(out=a_o, lhsT=selb_sb[:, 1, :], rhs=rhs)
                            # DVE may read at most one PSUM operand: a_o
                            # stays in PSUM, a_s comes via a staged copy.
                            # (gpsimd cannot access PSUM at all.)
                            ss = stg.tile([128, LQ], BF16, tag="ss")
                            nc.scalar.copy(ss, a_s)
                            nc.vector.tensor_mul(pmt[:, hp, :], ss, a_o)
                        # head-sum tree: pairwise plain adds (2x bf16 mode)
                        nc.vector.tensor_add(pmt[:, 0, :], pmt[:, 0, :], pmt[:, 1, :])
                        nc.gpsimd.tensor_add(pmt[:, 2, :], pmt[:, 2, :], pmt[:, 3, :])
                        nc.gpsimd.tensor_add(pmt[:, 4, :], pmt[:, 4, :], pmt[:, 5, :])
                        q2t = q2p.tile([128, LQ], BF16, tag="q2")
                        nc.vector.tensor_add(pmt[:, 0, :], pmt[:, 0, :], pmt[:, 2, :])
                        nc.vector.tensor_add(q2t, pmt[:, 0, :], pmt[:, 4, :])
                        # transpose q2 into l-major bf16 PSUM
                        qt_ps = psT.tile([128, 4, 128], BF16, tag="qt", bufs=1)
                        for k in range(4):
                            nc.tensor.matmul(
                                out=qt_ps[:, k, :],
                                lhsT=q2t[:, k * 128:(k + 1) * 128],
                                rhs=idb_sb, is_transpose=True)
                        # stage this lq's qT2 chunks to SBUF and run their
                        # context matmuls right away (spreads ctx + tail work)
                        if lq % 2 == 0:
                            nc.scalar.copy(qT2sb[:, lq * 4:lq * 4 + 4, :], qt_ps)
                        else:
                            nc.vector.tensor_copy(qT2sb[:, lq * 4:lq * 4 + 4, :], qt_ps)
                        for cp in (lq * 4, lq * 4 + 2):
                            nc.tensor.matmul(
                                out=c2a, lhsT=qT2sb[:, cp:cp + 2, :],
                                rhs=seq_sb[:, cp:cp + 2, 0:384],
                                start=(cp == 0), stop=(cp == 14),
                                perf_mode=mybir.MatmulPerfMode.DoubleRow)
                            nc.tensor.matmul(
                                out=c2b, lhsT=qT2sb[:, cp:cp + 2, :],
                                rhs=seq_sb[:, cp:cp + 2, 384:DSEQ],
                                start=(cp == 0), stop=(cp == 14),
                                perf_mode=mybir.MatmulPerfMode.DoubleRow)
                    # stage c2 out of PSUM (frees psC banks for the tail)
                    nc.scalar.copy(c2sb[:, 0:384], c2a)
                    nc.vector.tensor_copy(c2sb[:, 384:DSEQ], c2b)

            # Tail: fold, unnormalized c scores, then combine with the
            # entity scores scaled by 1/qsum.  <= 6 PSUM banks.
            with tc.tile_pool(name="psF", bufs=1, space="PSUM") as psF:
                # fold head-halves: c2 -> c (col 768 = qsum); c stays
                # unnormalized, the 1/qsum lands on the c-scores at the end.
                c_psa = psF.tile([NP, 512], F32, tag="ca")
                c_psb = psF.tile([NP, DSEQ - 512], F32, tag="cb")
                nc.tensor.matmul(out=c_psa, lhsT=fold2_sb, rhs=c2sb[:, 0:512])
                nc.tensor.matmul(out=c_psb, lhsT=fold2_sb, rhs=c2sb[:, 512:DSEQ])
                nc.scalar.copy(csb[:, 0:512], c_psa)
                nc.vector.tensor_copy(csb[:, 512:D], c_psb[:, 0:D - 512])
                nc.vector.reciprocal(rq, c_psb[:, D - 512:D - 512 + 1])
                cT_ps = psF.tile([128, 6, NP], BF16, tag="cT")
                for db in range(6):
                    nc.tensor.matmul(
                        out=cT_ps[:, db, :],
                        lhsT=csb[:, db * 128:(db + 1) * 128],
                        rhs=idb_sb[0:NP, 0:NP], is_transpose=True)
                nc.scalar.copy(embsT[:, 12:18, :], cT_ps)

                # c-part scores only (rel chunks 12..17), transpose, combine
                sc_ps = psF.tile([RN, NP], F32, tag="sc")
                for kc in range(12, 18):
                    nc.tensor.matmul(
                        out=sc_ps, lhsT=rel_sb[:, kc, :], rhs=embsT[:, kc, :],
                        start=(kc == 12), stop=(kc == 17))
                nc.scalar.copy(scsb, sc_ps)
                scT_ps = psF.tile([NP, RN], BF16, tag="scT")
                nc.tensor.matmul(out=scT_ps, lhsT=scsb, rhs=idb_sb[0:RN, 0:RN],
                                 is_transpose=True)
                ft = consts.tile([NP, RN], F32)
                nc.vector.scalar_tensor_tensor(
                    out=ft, in0=scT_ps, scalar=rq, in1=sceT,
                    op0=mult, op1=addop)
                nc.vector.reduce_max(nota_t, ft[:, R:RN],
                                     axis=mybir.AxisListType.X)
                nc.vector.tensor_copy(fin[:, 1:R + 1], ft[:, 0:R])
                nc.vector.tensor_copy(fin[:, 0:1], nota_t)

            nc.sync.dma_start(out=out[:, :], in_=fin)
            if debug:
                dq = consts.tile([128, 16, 128], F32)
                nc.vector.tensor_copy(dq, qT2sb)
                nc.sync.dma_start(out=dbg_q.rearrange("p (c n) -> p c n", c=16), in_=dq)
                dc2 = consts.tile([128, DSEQ], F32)
                nc.vector.tensor_copy(dc2, c2sb)
                nc.sync.dma_start(out=dbg_c2[:, :], in_=dc2)
                de = consts.tile([128, 18, NP], F32)
                nc.vector.tensor_copy(de, embsT)
                nc.sync.dma_start(out=dbg_emb.rearrange("p (c n) -> p c n", c=18), in_=de)

    return nc


def _in_maps(sequence_output, attention, relation_embeddings, nota_embeddings,
             span_starts):
    sequence_output = np.asarray(sequence_output, np.float32)
    attention = np.asarray(attention, np.float32)
    span_starts = np.asarray(span_starts)
    rel_t = np.ascontiguousarray(
        np.concatenate(
            [np.asarray(relation_embeddings, np.float32),
             np.asarray(nota_embeddings, np.float32)], axis=0
        ).T
    )
    rel_pm = rel_t.astype(NP_BF16).reshape(18, 128, RN).transpose(1, 0, 2)

    in_maps = []
    for c in range(NCORES):
        b, g = divmod(c, 4)
        ents = GROUP_ENTS[g]
        rows = np.concatenate(
            [np.arange(span_starts[b, e], span_starts[b, e] + W) for e in ents]
        )
        att_rows = attention[b][:, rows, :].reshape(H * NEW, L)
        sel_s, sel_o = _sel_matrices(g)
        att_q = att_rows.astype(NP_FP8).reshape(6, 128, 4, LQ).transpose(2, 1, 0, 3)
        seq_pm = np.zeros((128, 16, DSEQ), NP_FP8)
        seq_pm[:, :, 0:D] = sequence_output[b].astype(NP_FP8).reshape(16, 128, D).transpose(1, 0, 2)
        seq_pm[:, :, D] = 1.0
        def _bd(m):
            z = np.zeros_like(m)
            return np.block([[m, z], [z, m]])
        selb_h = np.concatenate([_bd(sel_s), _bd(sel_o)], axis=1)  # [128, 256]
        sele_h = np.concatenate([sel_s * 0.25, sel_o * 0.25], axis=1)  # [64, 128]
        in_maps.append({
            "att_g": np.ascontiguousarray(att_q.reshape(4, 128, 6 * LQ)),
            "seq": np.ascontiguousarray(seq_pm.reshape(128, 16 * DSEQ)),
            "spans": np.ascontiguousarray(sequence_output[b][rows].astype(NP_BF16)),
            "selb": np.ascontiguousarray(selb_h.astype(NP_FP8)),
            "sele": np.ascontiguousarray(sele_h.astype(NP_BF16)),
            "rel_t": np.ascontiguousarray(rel_pm.reshape(128, 18 * RN)),
        })
    return in_maps


def kernel(sequence_output, attention, relation_embeddings, nota_embeddings,
           span_starts):
    global LAST_RESULTS
    in_maps = _in_maps(sequence_output, attention, relation_embeddings,
                       nota_embeddings, span_starts)
    nc = _build_program()
    nc.finalize()  # Bacc legalization (wait splitting, reg alloc)
    LAST_RESULTS = run_bass_kernel_spmd(nc, in_maps, core_ids=list(range(NCORES)))

    out = np.zeros((B, len(ALL_PAIRS), R + 1), np.float32)
    for c in range(NCORES):
        b, g = divmod(c, 4)
        idxs = GROUP_IDX[g]
        out[b, idxs, :] = LAST_RESULTS.results[c]["out"][: len(idxs)]
    return out


# revision 39
# speedup vs baseline: 1.1756x; 1.0209x over previous
"""Trainium2 Bass kernel for the gnn_message_passing encoder problem.

kernel(**inputs) takes the FULL inputs and returns the FULL [B, P, R+1] output.

Sharding: 8 cores = 2 batches x 4 object-groups; each core scores 64 padded
(trigger, object) pairs of one document.  Host ships only the gathered
attention rows (bf16, [head-pair, (e,w), L] tiles), the full batch
sequence_output in L-chunk-major layout with a fused ones-column (so the
q row-sum rides the context matmul for free), span token rows, selector
matrices, and the transposed codebooks.

Device pipeline per core:
  1. Pair expansion on PE with block-diagonal one-hot selectors (W-sum is
     folded into the selectors; its scale cancels in the q/qsum ratio):
     two [128,512] matmuls per (head-pair, L-quarter) produce a_s / a_o
     in PSUM at M=128.
  2. a_s staged PSUM->SBUF bf16 on the scalar engine; products
     pm = a_s * a_o on vector (2/3) and gpsimd (1/3) engines.
  3. Head-sum tree on vector (scalar_tensor_tensor, all-SBUF bf16),
     leaving two head-half copies per partition-half (no fold yet).
  4. PE transposes q2 [128,128] chunks into bf16 PSUM (l on partitions),
     scalar engine copies them to SBUF; context matmuls accumulate
     c2 = q2T^T @ [seq | 1] over all 16 L-chunks (col 768 = qsum).
  5. Tail: fold the two head-halves of c2 with a [I;I] matmul, reciprocal
     of qsum, normalize c on the scalar engine (per-partition scale AP),
     transpose into the f-major embs layout, one 18-chunk scoring matmul
     against [rel; nota], transpose, NOTA max, output DMA.
"""

import os
import sys

import numpy as np

for _p in ("/opt/trn_rl_repo", os.path.expanduser("~/.axon_site/_ro/trn_rl_repo")):
    if os.path.isdir(_p) and _p not in sys.path:
        sys.path.insert(0, _p)

import concourse.bass as bass
import concourse.mybir as mybir
import concourse.tile as tile
from concourse import bacc
from concourse.bass_utils import run_bass_kernel_spmd

# Problem dimensions (hardcoded per the harness contract).
B, L, D, H = 2, 2048, 768, 12
E, T, W = 32, 8, 4
R, NN = 57, 20
RN = R + NN            # 77 stacked codebook rows
NE = 16                # entities per core (8 triggers + 8 objects)
NEW = NE * W           # 64 gathered rows per head
NP = 64                # pair slots per core (group 0 pads 56 -> 64)
LQ = 512               # L is processed in 4 slices of 512
DSEQ = D + 8           # seq free dim with ones column at 768 (pad to 776)
NCORES = 8

# Static pair list in the reference's order (s-major).
ALL_PAIRS = [(s, o) for s in range(T) for o in range(E) if s != o]
GROUP_IDX = [[i for i, (_, o) in enumerate(ALL_PAIRS) if o // 8 == g] for g in range(4)]
GROUP_ENTS = [
    list(range(16)),
    list(range(16)),
    list(range(8)) + list(range(16, 24)),
    list(range(8)) + list(range(24, 32)),
]

F32 = mybir.dt.float32
BF16 = mybir.dt.bfloat16
FP8 = mybir.dt.float8e4
import ml_dtypes
NP_BF16 = ml_dtypes.bfloat16
NP_FP8 = ml_dtypes.float8_e4m3

LAST_RESULTS = None  # BassKernelResults of the most recent kernel() call


def _sel_matrices(g):
    """Attention selectors (1.0; scale-free) and entity selectors (0.25)."""
    idxs = GROUP_IDX[g]
    ents = GROUP_ENTS[g]
    local = {e: i for i, e in enumerate(ents)}
    sel_s = np.zeros((NEW, NP), np.float32)
    sel_o = np.zeros((NEW, NP), np.float32)
    for j in range(NP):
        s, o = ALL_PAIRS[idxs[j % len(idxs)]]  # pad group 0 by repeating pair 0
        for w in range(W):
            sel_s[local[s] * W + w, j] = 1.0
            sel_o[local[o] * W + w, j] = 1.0
    return sel_s, sel_o


def _sel_doublerow(g):
    """DoubleRow expansion selectors [side, t, 128, 2, 128].

    k-tile partition p = h_loc*32 + e*2 + wg holds att rows (head 4*st +
    h_loc, entity e, w = 2*wg + j) in slot j.  Output partition m = hh*64 +
    pair covers heads (2t+hh) of the supertile; the W-sum spreads 1.0 over
    all four (wg, j) combinations (its scale cancels in q/qsum).
    """
    idxs = GROUP_IDX[g]
    ents = GROUP_ENTS[g]
    local = {e: i for i, e in enumerate(ents)}
    dr = np.zeros((2, 2, 128, 2, 128), np.float32)
    for j in range(NP):
        s, o = ALL_PAIRS[idxs[j % len(idxs)]]
        for side, ent in ((0, s), (1, o)):
            el = local[ent]
            for t in range(2):
                for hh in range(2):
                    m = hh * NP + j
                    h_loc = 2 * t + hh
                    for wg in range(2):
                        for js in range(2):
                            dr[side, t, h_loc * 32 + el * 2 + wg, js, m] = 1.0
    return dr


def _build_program(debug=False):
    nc = bacc.Bacc("TRN2")

    att_g = nc.dram_tensor("att_g", [4, 128, 6 * LQ], FP8, kind="ExternalInput")
    seq = nc.dram_tensor("seq", [128, 16 * DSEQ], FP8, kind="ExternalInput")
    spans = nc.dram_tensor("spans", [NEW, D], BF16, kind="ExternalInput")
    selb = nc.dram_tensor("selb", [128, 2 * 128], FP8, kind="ExternalInput")
    sele = nc.dram_tensor("sele", [NEW, 2 * NP], BF16, kind="ExternalInput")
    rel_t = nc.dram_tensor("rel_t", [128, 18 * RN], BF16, kind="ExternalInput")
    out = nc.dram_tensor("out", [NP, R + 1], F32, kind="ExternalOutput")
    if debug:
        dbg_q = nc.dram_tensor("dbg_q", [128, 16 * 128], F32, kind="ExternalOutput")
        dbg_c2 = nc.dram_tensor("dbg_c2", [128, DSEQ], F32, kind="ExternalOutput")
        dbg_emb = nc.dram_tensor("dbg_emb", [128, 18 * NP], F32, kind="ExternalOutput")

    mult = mybir.AluOpType.mult
    addop = mybir.AluOpType.add

    with tile.TileContext(nc) as tc:
        with tc.tile_pool(name="consts", bufs=1) as consts:
            # Small inputs first so the entity phase and expansion can start
            # immediately; seq is split in halves and interleaved between
            # attention quarters so the first context matmuls aren't starved.
            selb_sb = consts.tile([128, 2, 128], FP8)
            nc.sync.dma_start(out=selb_sb, in_=selb.rearrange("p (s n) -> p s n", s=2))
            sele_sb = consts.tile([NEW, 2, NP], BF16)
            nc.sync.dma_start(out=sele_sb, in_=sele.rearrange("p (s n) -> p s n", s=2))
            spans_sb = consts.tile([NEW, D], BF16)
            nc.sync.dma_start(out=spans_sb, in_=spans[:, :])
            g_sb = consts.tile([128, 6, L], FP8)
            g_view = att_g.rearrange("q p (t l) -> q p t l", t=6)
            nc.sync.dma_start(out=g_sb[:, :, 0:LQ], in_=g_view[0])
            nc.sync.dma_start(out=g_sb[:, :, LQ:2 * LQ], in_=g_view[1])
            seq_sb = consts.tile([128, 16, DSEQ], FP8)
            seq_view = seq.rearrange("p (c d) -> p c d", c=16)
            nc.sync.dma_start(out=seq_sb[:, 0:8, :], in_=seq_view[:, 0:8, :])
            nc.sync.dma_start(out=g_sb[:, :, 2 * LQ:3 * LQ], in_=g_view[2])
            nc.sync.dma_start(out=seq_sb[:, 8:16, :], in_=seq_view[:, 8:16, :])
            nc.sync.dma_start(out=g_sb[:, :, 3 * LQ:4 * LQ], in_=g_view[3])
            rel_sb = consts.tile([128, 18, RN], BF16)
            nc.sync.dma_start(out=rel_sb, in_=rel_t.rearrange("p (c n) -> p c n", c=18))

            # fold2 = [I64; I64] and a bf16 identity, built on device.
            fold2_sb = consts.tile([128, NP], BF16)
            nc.gpsimd.memset(fold2_sb, 0.0)
            nc.gpsimd.affine_select(
                out=fold2_sb[0:NP, :], in_=fold2_sb[0:NP, :],
                compare_op=mybir.AluOpType.not_equal, fill=1.0, base=0,
                pattern=[[-1, NP]], channel_multiplier=1,
            )
            nc.gpsimd.affine_select(
                out=fold2_sb[NP:128, :], in_=fold2_sb[NP:128, :],
                compare_op=mybir.AluOpType.not_equal, fill=1.0, base=0,
                pattern=[[-1, NP]], channel_multiplier=1,
            )
            idb_sb = consts.tile([128, 128], BF16)
            nc.gpsimd.memset(idb_sb, 0.0)
            nc.gpsimd.affine_select(
                out=idb_sb, in_=idb_sb,
                compare_op=mybir.AluOpType.not_equal, fill=1.0, base=0,
                pattern=[[-1, 128]], channel_multiplier=1,
            )

            qT2sb = consts.tile([128, 16, 128], FP8)
            embsT = consts.tile([128, 18, NP], BF16)
            c2sb = consts.tile([128, DSEQ], BF16)
            csb = consts.tile([NP, D], BF16)
            scsb = consts.tile([RN, NP], BF16)
            rq = consts.tile([NP, 1], F32)
            nota_t = consts.tile([NP, 1], F32)
            fin = consts.tile([NP, R + 1], F32)

            # Entity phase first: it only needs spans+sele+rel, so it runs
            # while the attention quarters are still streaming in.  The
            # entity part of the scores (rel chunks 0..11) is also fully
            # computed here; the c-part joins at the tail.  Its PSUM pool
            # closes before the main pipeline's pools open.
            sceT = consts.tile([NP, RN], BF16)
            with tc.tile_pool(name="psE", bufs=1, space="PSUM") as psE:
                e_ps0 = psE.tile([128, 3, 128], F32, tag="e0")
                e_ps1 = psE.tile([128, 3, 128], F32, tag="e1")
                for db in range(6):
                    e_ps = e_ps0 if db < 3 else e_ps1
                    nc.tensor.matmul(
                        out=e_ps[:, db % 3, :],
                        lhsT=spans_sb[:, db * 128:(db + 1) * 128],
                        rhs=sele_sb.rearrange("p s n -> p (s n)"))
                nc.scalar.copy(embsT[:, 0:3, :], e_ps0[:, :, 0:NP])
                nc.vector.tensor_copy(embsT[:, 6:9, :], e_ps0[:, :, NP:128])
                nc.scalar.copy(embsT[:, 3:6, :], e_ps1[:, :, 0:NP])
                nc.vector.tensor_copy(embsT[:, 9:12, :], e_ps1[:, :, NP:128])


            # Main pipeline: PSUM = psA(2+3=5) + psT(2x0.5=1) + psC(2) = 8 banks.
            with tc.tile_pool(name="psT", bufs=1, space="PSUM") as psT, \
                 tc.tile_pool(name="psC", bufs=1, space="PSUM") as psC:
                c2a = psC.tile([128, 384], F32, tag="c2a")
                c2b = psC.tile([128, DSEQ - 384], F32, tag="c2b")

                with tc.tile_pool(name="psA", bufs=2, space="PSUM") as psA, \
                     tc.tile_pool(name="prod", bufs=2) as prod, \
                     tc.tile_pool(name="stg", bufs=3) as stg, \
                     tc.tile_pool(name="q2p", bufs=2) as q2p:
                    for lq in range(4):
                        pmt = prod.tile([128, 6, LQ], BF16, tag="pm")
                        for hp in range(6):
                            u = lq * 6 + hp
                            a_s = psA.tile([128, LQ], F32, tag="as", bufs=2)
                            a_o = psA.tile([128, LQ], F32, tag="ao", bufs=3)
                            rhs = g_sb[:, hp, lq * LQ:(lq + 1) * LQ]
                            nc.tensor.matmul(out=a_s, lhsT=selb_sb[:, 0, :], rhs=rhs)
                            nc.tensor.matmul(out=a_o, lhsT=selb_sb[:, 1, :], rhs=rhs)
                            # DVE may read at most one PSUM operand: a_o
                            # stays in PSUM, a_s comes via a staged copy.
                            # (gpsimd cannot access PSUM at all.)
                            ss = stg.tile([128, LQ], BF16, tag="ss")
                            nc.scalar.copy(ss, a_s)
                            nc.vector.tensor_mul(pmt[:, hp, :], ss, a_o)
                        # head-sum tree: pairwise plain adds (2x bf16 mode)
                        nc.vector.tensor_add(pmt[:, 0, :], pmt[:, 0, :], pmt[:, 1, :])
                        nc.gpsimd.tensor_add(pmt[:, 2, :], pmt[:, 2, :], pmt[:, 3, :])
                        nc.gpsimd.tensor_add(pmt[:, 4, :], pmt[:, 4, :], pmt[:, 5, :])
                        q2t = q2p.tile([128, LQ], BF16, tag="q2")
                        nc.vector.tensor_add(pmt[:, 0, :], pmt[:, 0, :], pmt[:, 2, :])
                        nc.vector.tensor_add(q2t, pmt[:, 0, :], pmt[:, 4, :])
                        # transpose q2 into l-major bf16 PSUM
                        qt_ps = psT.tile([128, 4, 128], BF16, tag="qt", bufs=1)
                        for k in range(4):
                            nc.tensor.matmul(
                                out=qt_ps[:, k, :],
                                lhsT=q2t[:, k * 128:(k + 1) * 128],
                                rhs=idb_sb, is_transpose=True)
                        # stage this lq's qT2 chunks to SBUF and run their
                        # context matmuls right away (spreads ctx + tail work)
                        if lq % 2 == 0:
                            nc.scalar.copy(qT2sb[:, lq * 4:lq * 4 + 4, :], qt_ps)
                        else:
                            nc.vector.tensor_copy(qT2sb[:, lq * 4:lq * 4 + 4, :], qt_ps)
                        for cp in (lq * 4, lq * 4 + 2):
                            nc.tensor.matmul(
                                out=c2a, lhsT=qT2sb[:, cp:cp + 2, :],
                                rhs=seq_sb[:, cp:cp + 2, 0:384],
                                start=(cp == 0), stop=(cp == 14),
                                perf_mode=mybir.MatmulPerfMode.DoubleRow)
                            nc.tensor.matmul(
                                out=c2b, lhsT=qT2sb[:, cp:cp + 2, :],
                                rhs=seq_sb[:, cp:cp + 2, 384:DSEQ],
                                start=(cp == 0), stop=(cp == 14),
                                perf_mode=mybir.MatmulPerfMode.DoubleRow)
                    # stage c2 out of PSUM (frees psC banks for the tail)
                    nc.scalar.copy(c2sb[:, 0:384], c2a)
                    nc.vector.tensor_copy(c2sb[:, 384:DSEQ], c2b)

            # Tail: fold, unnormalized c scores, then combine with the
            # entity scores scaled by 1/qsum.  <= 6 PSUM banks.
            with tc.tile_pool(name="psF", bufs=1, space="PSUM") as psF:
                # entity-part scores first: PE runs these while Act/DVE drain
                # the c2 copies.
                sce_ps = psF.tile([RN, NP], F32, tag="sce")
                for kc in range(12):
                    nc.tensor.matmul(
                        out=sce_ps, lhsT=rel_sb[:, kc, :], rhs=embsT[:, kc, :],
                        start=(kc == 0), stop=(kc == 11))
                scesb = consts.tile([RN, NP], BF16)
                nc.scalar.copy(scesb, sce_ps)
                sceT_ps = psF.tile([NP, RN], BF16, tag="sceT")
                nc.tensor.matmul(out=sceT_ps, lhsT=scesb, rhs=idb_sb[0:RN, 0:RN],
                                 is_transpose=True)
                nc.vector.tensor_copy(sceT, sceT_ps)
                # fold head-halves: c2 -> c (col 768 = qsum); c stays
                # unnormalized, the 1/qsum lands on the c-scores at the end.
                c_psa = psF.tile([NP, 512], F32, tag="ca")
                c_psb = psF.tile([NP, DSEQ - 512], F32, tag="cb")
                nc.tensor.matmul(out=c_psa, lhsT=fold2_sb, rhs=c2sb[:, 0:512])
                nc.tensor.matmul(out=c_psb, lhsT=fold2_sb, rhs=c2sb[:, 512:DSEQ])
                nc.scalar.copy(csb[:, 0:512], c_psa)
                nc.vector.tensor_copy(csb[:, 512:D], c_psb[:, 0:D - 512])
                nc.vector.reciprocal(rq, c_psb[:, D - 512:D - 512 + 1])
                cT_ps = psF.tile([128, 6, NP], BF16, tag="cT")
                for db in range(6):
                    nc.tensor.matmul(
                        out=cT_ps[:, db, :],
                        lhsT=csb[:, db * 128:(db + 1) * 128],
                        rhs=idb_sb[0:NP, 0:NP], is_transpose=True)
                nc.scalar.copy(embsT[:, 12:18, :], cT_ps)

                # c-part scores only (rel chunks 12..17), transpose, combine
                sc_ps = psF.tile([RN, NP], F32, tag="sc")
                for kc in range(12, 18):
                    nc.tensor.matmul(
                        out=sc_ps, lhsT=rel_sb[:, kc, :], rhs=embsT[:, kc, :],
                        start=(kc == 12), stop=(kc == 17))
                nc.scalar.copy(scsb, sc_ps)
                scT_ps = psF.tile([NP, RN], BF16, tag="scT")
                nc.tensor.matmul(out=scT_ps, lhsT=scsb, rhs=idb_sb[0:RN, 0:RN],
                                 is_transpose=True)
                ft = consts.tile([NP, RN], F32)
                nc.vector.scalar_tensor_tensor(
                    out=ft, in0=scT_ps, scalar=rq, in1=sceT,
                    op0=mult, op1=addop)
                nc.vector.reduce_max(nota_t, ft[:, R:RN],
                                     axis=mybir.AxisListType.X)
                nc.vector.tensor_copy(fin[:, 1:R + 1], ft[:, 0:R])
                nc.vector.tensor_copy(fin[:, 0:1], nota_t)

            nc.sync.dma_start(out=out[:, :], in_=fin)
            if debug:
                dq = consts.tile([128, 16, 128], F32)
                nc.vector.tensor_copy(dq, qT2sb)
                nc.sync.dma_start(out=dbg_q.rearrange("p (c n) -> p c n", c=16), in_=dq)
                dc2 = consts.tile([128, DSEQ], F32)
                nc.vector.tensor_copy(dc2, c2sb)
                nc.sync.dma_start(out=dbg_c2[:, :], in_=dc2)
                de = consts.tile([128, 18, NP], F32)
                nc.vector.tensor_copy(de, embsT)
                nc.sync.dma_start(out=dbg_emb.rearrange("p (c n) -> p c n", c=18), in_=de)

    return nc


def _in_maps(sequence_output, attention, relation_embeddings, nota_embeddings,
             span_starts):
    sequence_output = np.asarray(sequence_output, np.float32)
    attention = np.asarray(attention, np.float32)
    span_starts = np.asarray(span_starts)
    rel_t = np.ascontiguousarray(
        np.concatenate(
            [np.asarray(relation_embeddings, np.float32),
             np.asarray(nota_embeddings, np.float32)], axis=0
        ).T
    )
    rel_pm = rel_t.astype(NP_BF16).reshape(18, 128, RN).transpose(1, 0, 2)

    in_maps = []
    for c in range(NCORES):
        b, g = divmod(c, 4)
        ents = GROUP_ENTS[g]
        rows = np.concatenate(
            [np.arange(span_starts[b, e], span_starts[b, e] + W) for e in ents]
        )
        att_rows = attention[b][:, rows, :].reshape(H * NEW, L)
        sel_s, sel_o = _sel_matrices(g)
        att_q = att_rows.astype(NP_FP8).reshape(6, 128, 4, LQ).transpose(2, 1, 0, 3)
        seq_pm = np.zeros((128, 16, DSEQ), NP_FP8)
        seq_pm[:, :, 0:D] = sequence_output[b].astype(NP_FP8).reshape(16, 128, D).transpose(1, 0, 2)
        seq_pm[:, :, D] = 1.0
        def _bd(m):
            z = np.zeros_like(m)
            return np.block([[m, z], [z, m]])
        selb_h = np.concatenate([_bd(sel_s), _bd(sel_o)], axis=1)  # [128, 256]
        sele_h = np.concatenate([sel_s * 0.25, sel_o * 0.25], axis=1)  # [64, 128]
        in_maps.append({
            "att_g": np.ascontiguousarray(att_q.reshape(4, 128, 6 * LQ)),
            "seq": np.ascontiguousarray(seq_pm.reshape(128, 16 * DSEQ)),
            "spans": np.ascontiguousarray(sequence_output[b][rows].astype(NP_BF16)),
            "selb": np.ascontiguousarray(selb_h.astype(NP_FP8)),
            "sele": np.ascontiguousarray(sele_h.astype(NP_BF16)),
            "rel_t": np.ascontiguousarray(rel_pm.reshape(128, 18 * RN)),
        })
    return in_maps


def kernel(sequence_output, attention, relation_embeddings, nota_embeddings,
           span_starts):
    global LAST_RESULTS
    in_maps = _in_maps(sequence_output, attention, relation_embeddings,
                       nota_embeddings, span_starts)
    nc = _build_program()
    nc.finalize()  # Bacc legalization (wait splitting, reg alloc)
    LAST_RESULTS = run_bass_kernel_spmd(nc, in_maps, core_ids=list(range(NCORES)))

    out = np.zeros((B, len(ALL_PAIRS), R + 1), np.float32)
    for c in range(NCORES):
        b, g = divmod(c, 4)
        idxs = GROUP_IDX[g]
        out[b, idxs, :] = LAST_RESULTS.results[c]["out"][: len(idxs)]
    return out


# revision 40
# speedup vs baseline: 1.1955x; 1.0169x over previous
"""Trainium2 Bass kernel for the gnn_message_passing encoder problem.

kernel(**inputs) takes the FULL inputs and returns the FULL [B, P, R+1] output.

Sharding: 8 cores = 2 batches x 4 object-groups; each core scores 64 padded
(trigger, object) pairs of one document.  Host ships only the gathered
attention rows (bf16, [head-pair, (e,w), L] tiles), the full batch
sequence_output in L-chunk-major layout with a fused ones-column (so the
q row-sum rides the context matmul for free), span token rows, selector
matrices, and the transposed codebooks.

Device pipeline per core:
  1. Pair expansion on PE with block-diagonal one-hot selectors (W-sum is
     folded into the selectors; its scale cancels in the q/qsum ratio):
     two [128,512] matmuls per (head-pair, L-quarter) produce a_s / a_o
     in PSUM at M=128.
  2. a_s staged PSUM->SBUF bf16 on the scalar engine; products
     pm = a_s * a_o on vector (2/3) and gpsimd (1/3) engines.
  3. Head-sum tree on vector (scalar_tensor_tensor, all-SBUF bf16),
     leaving two head-half copies per partition-half (no fold yet).
  4. PE transposes q2 [128,128] chunks into bf16 PSUM (l on partitions),
     scalar engine copies them to SBUF; context matmuls accumulate
     c2 = q2T^T @ [seq | 1] over all 16 L-chunks (col 768 = qsum).
  5. Tail: fold the two head-halves of c2 with a [I;I] matmul, reciprocal
     of qsum, normalize c on the scalar engine (per-partition scale AP),
     transpose into the f-major embs layout, one 18-chunk scoring matmul
     against [rel; nota], transpose, NOTA max, output DMA.
"""

import os
import sys

import numpy as np

for _p in ("/opt/trn_rl_repo", os.path.expanduser("~/.axon_site/_ro/trn_rl_repo")):
    if os.path.isdir(_p) and _p not in sys.path:
        sys.path.insert(0, _p)

import concourse.bass as bass
import concourse.mybir as mybir
import concourse.tile as tile
from concourse import bacc
from concourse.bass_utils import run_bass_kernel_spmd

# Problem dimensions (hardcoded per the harness contract).
B, L, D, H = 2, 2048, 768, 12
E, T, W = 32, 8, 4
R, NN = 57, 20
RN = R + NN            # 77 stacked codebook rows
NE = 16                # entities per core (8 triggers + 8 objects)
NEW = NE * W           # 64 gathered rows per head
NP = 64                # pair slots per core (group 0 pads 56 -> 64)
LQ = 512               # L is processed in 4 slices of 512
DSEQ = D + 8           # seq free dim with ones column at 768 (pad to 776)
NCORES = 8

# Static pair list in the reference's order (s-major).
ALL_PAIRS = [(s, o) for s in range(T) for o in range(E) if s != o]
GROUP_IDX = [[i for i, (_, o) in enumerate(ALL_PAIRS) if o // 8 == g] for g in range(4)]
GROUP_ENTS = [
    list(range(16)),
    list(range(16)),
    list(range(8)) + list(range(16, 24)),
    list(range(8)) + list(range(24, 32)),
]

F32 = mybir.dt.float32
BF16 = mybir.dt.bfloat16
FP8 = mybir.dt.float8e4
import ml_dtypes
NP_BF16 = ml_dtypes.bfloat16
NP_FP8 = ml_dtypes.float8_e4m3

LAST_RESULTS = None  # BassKernelResults of the most recent kernel() call


def _sel_matrices(g):
    """Attention selectors (1.0; scale-free) and entity selectors (0.25)."""
    idxs = GROUP_IDX[g]
    ents = GROUP_ENTS[g]
    local = {e: i for i, e in enumerate(ents)}
    sel_s = np.zeros((NEW, NP), np.float32)
    sel_o = np.zeros((NEW, NP), np.float32)
    for j in range(NP):
        s, o = ALL_PAIRS[idxs[j % len(idxs)]]  # pad group 0 by repeating pair 0
        for w in range(W):
            sel_s[local[s] * W + w, j] = 1.0
            sel_o[local[o] * W + w, j] = 1.0
    return sel_s, sel_o


def _sel_doublerow(g):
    """DoubleRow expansion selectors [side, t, 128, 2, 128].

    k-tile partition p = h_loc*32 + e*2 + wg holds att rows (head 4*st +
    h_loc, entity e, w = 2*wg + j) in slot j.  Output partition m = hh*64 +
    pair covers heads (2t+hh) of the supertile; the W-sum spreads 1.0 over
    all four (wg, j) combinations (its scale cancels in q/qsum).
    """
    idxs = GROUP_IDX[g]
    ents = GROUP_ENTS[g]
    local = {e: i for i, e in enumerate(ents)}
    dr = np.zeros((2, 2, 128, 2, 128), np.float32)
    for j in range(NP):
        s, o = ALL_PAIRS[idxs[j % len(idxs)]]
        for side, ent in ((0, s), (1, o)):
            el = local[ent]
            for t in range(2):
                for hh in range(2):
                    m = hh * NP + j
                    h_loc = 2 * t + hh
                    for wg in range(2):
                        for js in range(2):
                            dr[side, t, h_loc * 32 + el * 2 + wg, js, m] = 1.0
    return dr


def _build_program(debug=False):
    nc = bacc.Bacc("TRN2")

    att_g = nc.dram_tensor("att_g", [4, 128, 6 * LQ], FP8, kind="ExternalInput")
    seq = nc.dram_tensor("seq", [128, 16 * DSEQ], FP8, kind="ExternalInput")
    spans = nc.dram_tensor("spans", [NEW, D], BF16, kind="ExternalInput")
    selb = nc.dram_tensor("selb", [128, 2 * 128], FP8, kind="ExternalInput")
    sele = nc.dram_tensor("sele", [NEW, 2 * NP], BF16, kind="ExternalInput")
    rel_t = nc.dram_tensor("rel_t", [128, 18 * RN], BF16, kind="ExternalInput")
    out = nc.dram_tensor("out", [NP, R + 1], F32, kind="ExternalOutput")
    if debug:
        dbg_q = nc.dram_tensor("dbg_q", [128, 16 * 128], F32, kind="ExternalOutput")
        dbg_c2 = nc.dram_tensor("dbg_c2", [128, DSEQ], F32, kind="ExternalOutput")
        dbg_emb = nc.dram_tensor("dbg_emb", [128, 18 * NP], F32, kind="ExternalOutput")

    mult = mybir.AluOpType.mult
    addop = mybir.AluOpType.add

    with tile.TileContext(nc) as tc:
        with tc.tile_pool(name="consts", bufs=1) as consts:
            # Small inputs first so the entity phase and expansion can start
            # immediately; seq is split in halves and interleaved between
            # attention quarters so the first context matmuls aren't starved.
            selb_sb = consts.tile([128, 2, 128], FP8)
            nc.sync.dma_start(out=selb_sb, in_=selb.rearrange("p (s n) -> p s n", s=2))
            sele_sb = consts.tile([NEW, 2, NP], BF16)
            nc.sync.dma_start(out=sele_sb, in_=sele.rearrange("p (s n) -> p s n", s=2))
            spans_sb = consts.tile([NEW, D], BF16)
            nc.sync.dma_start(out=spans_sb, in_=spans[:, :])
            g_sb = consts.tile([128, 6, L], FP8)
            g_view = att_g.rearrange("q p (t l) -> q p t l", t=6)
            nc.sync.dma_start(out=g_sb[:, :, 0:LQ], in_=g_view[0])
            nc.sync.dma_start(out=g_sb[:, :, LQ:2 * LQ], in_=g_view[1])
            seq_sb = consts.tile([128, 16, DSEQ], FP8)
            seq_view = seq.rearrange("p (c d) -> p c d", c=16)
            nc.sync.dma_start(out=seq_sb[:, 0:8, :], in_=seq_view[:, 0:8, :])
            nc.sync.dma_start(out=g_sb[:, :, 2 * LQ:3 * LQ], in_=g_view[2])
            nc.sync.dma_start(out=seq_sb[:, 8:16, :], in_=seq_view[:, 8:16, :])
            nc.sync.dma_start(out=g_sb[:, :, 3 * LQ:4 * LQ], in_=g_view[3])
            rel_sb = consts.tile([128, 18, RN], BF16)
            nc.sync.dma_start(out=rel_sb, in_=rel_t.rearrange("p (c n) -> p c n", c=18))

            # fold2 = [I64; I64] and a bf16 identity, built on device.
            fold2_sb = consts.tile([128, NP], BF16)
            nc.gpsimd.memset(fold2_sb, 0.0)
            nc.gpsimd.affine_select(
                out=fold2_sb[0:NP, :], in_=fold2_sb[0:NP, :],
                compare_op=mybir.AluOpType.not_equal, fill=1.0, base=0,
                pattern=[[-1, NP]], channel_multiplier=1,
            )
            nc.gpsimd.affine_select(
                out=fold2_sb[NP:128, :], in_=fold2_sb[NP:128, :],
                compare_op=mybir.AluOpType.not_equal, fill=1.0, base=0,
                pattern=[[-1, NP]], channel_multiplier=1,
            )
            idb_sb = consts.tile([128, 128], BF16)
            nc.gpsimd.memset(idb_sb, 0.0)
            nc.gpsimd.affine_select(
                out=idb_sb, in_=idb_sb,
                compare_op=mybir.AluOpType.not_equal, fill=1.0, base=0,
                pattern=[[-1, 128]], channel_multiplier=1,
            )

            qT2sb = consts.tile([128, 16, 128], FP8)
            embsT = consts.tile([128, 18, NP], BF16)
            c2sb = consts.tile([128, DSEQ], BF16)
            csb = consts.tile([NP, D], BF16)
            scsb = consts.tile([RN, NP], BF16)
            rq = consts.tile([NP, 1], F32)
            nota_t = consts.tile([NP, 1], F32)
            fin = consts.tile([NP, R + 1], F32)

            # Entity phase first: it only needs spans+sele+rel, so it runs
            # while the attention quarters are still streaming in.  The
            # entity part of the scores (rel chunks 0..11) is also fully
            # computed here; the c-part joins at the tail.  Its PSUM pool
            # closes before the main pipeline's pools open.
            with tc.tile_pool(name="psE", bufs=1, space="PSUM") as psE:
                e_ps0 = psE.tile([128, 3, 128], F32, tag="e0")
                e_ps1 = psE.tile([128, 3, 128], F32, tag="e1")
                for db in range(6):
                    e_ps = e_ps0 if db < 3 else e_ps1
                    nc.tensor.matmul(
                        out=e_ps[:, db % 3, :],
                        lhsT=spans_sb[:, db * 128:(db + 1) * 128],
                        rhs=sele_sb.rearrange("p s n -> p (s n)"))
                nc.scalar.copy(embsT[:, 0:3, :], e_ps0[:, :, 0:NP])
                nc.vector.tensor_copy(embsT[:, 6:9, :], e_ps0[:, :, NP:128])
                nc.scalar.copy(embsT[:, 3:6, :], e_ps1[:, :, 0:NP])
                nc.vector.tensor_copy(embsT[:, 9:12, :], e_ps1[:, :, NP:128])


            # Main pipeline: PSUM = psA(2+3=5) + psT(2x0.5=1) + psC(2) = 8 banks.
            with tc.tile_pool(name="psT", bufs=1, space="PSUM") as psT, \
                 tc.tile_pool(name="psC", bufs=1, space="PSUM") as psC:
                c2a = psC.tile([128, 384], F32, tag="c2a")
                c2b = psC.tile([128, DSEQ - 384], F32, tag="c2b")

                with tc.tile_pool(name="psA", bufs=2, space="PSUM") as psA, \
                     tc.tile_pool(name="prod", bufs=2) as prod, \
                     tc.tile_pool(name="stg", bufs=3) as stg, \
                     tc.tile_pool(name="q2p", bufs=2) as q2p:
                    for lq in range(4):
                        pmt = prod.tile([128, 6, LQ], BF16, tag="pm")
                        for hp in range(6):
                            u = lq * 6 + hp
                            a_s = psA.tile([128, LQ], F32, tag="as", bufs=2)
                            a_o = psA.tile([128, LQ], F32, tag="ao", bufs=3)
                            rhs = g_sb[:, hp, lq * LQ:(lq + 1) * LQ]
                            nc.tensor.matmul(out=a_s, lhsT=selb_sb[:, 0, :], rhs=rhs)
                            nc.tensor.matmul(out=a_o, lhsT=selb_sb[:, 1, :], rhs=rhs)
                            # DVE may read at most one PSUM operand: a_o
                            # stays in PSUM, a_s comes via a staged copy.
                            # (gpsimd cannot access PSUM at all.)
                            ss = stg.tile([128, LQ], BF16, tag="ss")
                            nc.scalar.copy(ss, a_s)
                            nc.vector.tensor_mul(pmt[:, hp, :], ss, a_o)
                        # head-sum tree: pairwise plain adds (2x bf16 mode)
                        nc.vector.tensor_add(pmt[:, 0, :], pmt[:, 0, :], pmt[:, 1, :])
                        nc.gpsimd.tensor_add(pmt[:, 2, :], pmt[:, 2, :], pmt[:, 3, :])
                        nc.gpsimd.tensor_add(pmt[:, 4, :], pmt[:, 4, :], pmt[:, 5, :])
                        q2t = q2p.tile([128, LQ], BF16, tag="q2")
                        nc.vector.tensor_add(pmt[:, 0, :], pmt[:, 0, :], pmt[:, 2, :])
                        nc.vector.tensor_add(q2t, pmt[:, 0, :], pmt[:, 4, :])
                        # transpose q2 into l-major bf16 PSUM
                        qt_ps = psT.tile([128, 4, 128], BF16, tag="qt", bufs=1)
                        for k in range(4):
                            nc.tensor.matmul(
                                out=qt_ps[:, k, :],
                                lhsT=q2t[:, k * 128:(k + 1) * 128],
                                rhs=idb_sb, is_transpose=True)
                        # stage this lq's qT2 chunks to SBUF and run their
                        # context matmuls right away (spreads ctx + tail work)
                        if lq % 2 == 0:
                            nc.scalar.copy(qT2sb[:, lq * 4:lq * 4 + 4, :], qt_ps)
                        else:
                            nc.vector.tensor_copy(qT2sb[:, lq * 4:lq * 4 + 4, :], qt_ps)
                        for cp in (lq * 4, lq * 4 + 2):
                            nc.tensor.matmul(
                                out=c2a, lhsT=qT2sb[:, cp:cp + 2, :],
                                rhs=seq_sb[:, cp:cp + 2, 0:384],
                                start=(cp == 0), stop=(cp == 14),
                                perf_mode=mybir.MatmulPerfMode.DoubleRow)
                            nc.tensor.matmul(
                                out=c2b, lhsT=qT2sb[:, cp:cp + 2, :],
                                rhs=seq_sb[:, cp:cp + 2, 384:DSEQ],
                                start=(cp == 0), stop=(cp == 14),
                                perf_mode=mybir.MatmulPerfMode.DoubleRow)
                    # stage c2 out of PSUM (frees psC banks for the tail)
                    nc.scalar.copy(c2sb[:, 0:384], c2a)
                    nc.vector.tensor_copy(c2sb[:, 384:DSEQ], c2b)

            # Tail: fold, unnormalized c scores, then combine with the
            # entity scores scaled by 1/qsum.  <= 6 PSUM banks.
            with tc.tile_pool(name="psF", bufs=1, space="PSUM") as psF:
                # fold head-halves: c = fold2^T @ c2  (col 768 = qsum)
                c_psa = psF.tile([NP, 512], F32, tag="ca")
                c_psb = psF.tile([NP, DSEQ - 512], F32, tag="cb")
                nc.tensor.matmul(out=c_psa, lhsT=fold2_sb, rhs=c2sb[:, 0:512])
                nc.tensor.matmul(out=c_psb, lhsT=fold2_sb, rhs=c2sb[:, 512:DSEQ])
                nc.vector.reciprocal(rq, c_psb[:, D - 512:D - 512 + 1])
                nc.scalar.mul(csb[:, 0:512], c_psa, rq)
                nc.vector.tensor_scalar_mul(csb[:, 512:D], c_psb[:, 0:D - 512], rq)
                cT_ps = psF.tile([128, 6, NP], BF16, tag="cT")
                for db in range(6):
                    nc.tensor.matmul(
                        out=cT_ps[:, db, :],
                        lhsT=csb[:, db * 128:(db + 1) * 128],
                        rhs=idb_sb[0:NP, 0:NP], is_transpose=True)
                nc.scalar.copy(embsT[:, 12:18, :], cT_ps)

                # scores = [rel; nota] @ embs, transpose, NOTA max
                sc_ps = psF.tile([RN, NP], F32, tag="sc")
                for kc in range(18):
                    nc.tensor.matmul(
                        out=sc_ps, lhsT=rel_sb[:, kc, :], rhs=embsT[:, kc, :],
                        start=(kc == 0), stop=(kc == 17))
                nc.scalar.copy(scsb, sc_ps)
                scT_ps = psF.tile([NP, RN], BF16, tag="scT")
                nc.tensor.matmul(out=scT_ps, lhsT=scsb, rhs=idb_sb[0:RN, 0:RN],
                                 is_transpose=True)
                nc.vector.reduce_max(nota_t, scT_ps[:, R:RN],
                                     axis=mybir.AxisListType.X)
                nc.vector.tensor_copy(fin[:, 1:R + 1], scT_ps[:, 0:R])
                nc.vector.tensor_copy(fin[:, 0:1], nota_t)

            nc.sync.dma_start(out=out[:, :], in_=fin)
            if debug:
                dq = consts.tile([128, 16, 128], F32)
                nc.vector.tensor_copy(dq, qT2sb)
                nc.sync.dma_start(out=dbg_q.rearrange("p (c n) -> p c n", c=16), in_=dq)
                dc2 = consts.tile([128, DSEQ], F32)
                nc.vector.tensor_copy(dc2, c2sb)
                nc.sync.dma_start(out=dbg_c2[:, :], in_=dc2)
                de = consts.tile([128, 18, NP], F32)
                nc.vector.tensor_copy(de, embsT)
                nc.sync.dma_start(out=dbg_emb.rearrange("p (c n) -> p c n", c=18), in_=de)

    return nc


def _in_maps(sequence_output, attention, relation_embeddings, nota_embeddings,
             span_starts):
    sequence_output = np.asarray(sequence_output, np.float32)
    attention = np.asarray(attention, np.float32)
    span_starts = np.asarray(span_starts)
    rel_t = np.ascontiguousarray(
        np.concatenate(
            [np.asarray(relation_embeddings, np.float32),
             np.asarray(nota_embeddings, np.float32)], axis=0
        ).T
    )
    rel_pm = rel_t.astype(NP_BF16).reshape(18, 128, RN).transpose(1, 0, 2)

    in_maps = []
    for c in range(NCORES):
        b, g = divmod(c, 4)
        ents = GROUP_ENTS[g]
        rows = np.concatenate(
            [np.arange(span_starts[b, e], span_starts[b, e] + W) for e in ents]
        )
        att_rows = attention[b][:, rows, :].reshape(H * NEW, L)
        sel_s, sel_o = _sel_matrices(g)
        att_q = att_rows.astype(NP_FP8).reshape(6, 128, 4, LQ).transpose(2, 1, 0, 3)
        seq_pm = np.zeros((128, 16, DSEQ), NP_FP8)
        seq_pm[:, :, 0:D] = sequence_output[b].astype(NP_FP8).reshape(16, 128, D).transpose(1, 0, 2)
        seq_pm[:, :, D] = 1.0
        def _bd(m):
            z = np.zeros_like(m)
            return np.block([[m, z], [z, m]])
        selb_h = np.concatenate([_bd(sel_s), _bd(sel_o)], axis=1)  # [128, 256]
        sele_h = np.concatenate([sel_s * 0.25, sel_o * 0.25], axis=1)  # [64, 128]
        in_maps.append({
            "att_g": np.ascontiguousarray(att_q.reshape(4, 128, 6 * LQ)),
            "seq": np.ascontiguousarray(seq_pm.reshape(128, 16 * DSEQ)),
            "spans": np.ascontiguousarray(sequence_output[b][rows].astype(NP_BF16)),
            "selb": np.ascontiguousarray(selb_h.astype(NP_FP8)),
            "sele": np.ascontiguousarray(sele_h.astype(NP_BF16)),
            "rel_t": np.ascontiguousarray(rel_pm.reshape(128, 18 * RN)),
        })
    return in_maps


def kernel(sequence_output, attention, relation_embeddings, nota_embeddings,
           span_starts):
    global LAST_RESULTS
    in_maps = _in_maps(sequence_output, attention, relation_embeddings,
                       nota_embeddings, span_starts)
    nc = _build_program()
    nc.finalize()  # Bacc legalization (wait splitting, reg alloc)
    LAST_RESULTS = run_bass_kernel_spmd(nc, in_maps, core_ids=list(range(NCORES)))

    out = np.zeros((B, len(ALL_PAIRS), R + 1), np.float32)
    for c in range(NCORES):
        b, g = divmod(c, 4)
        idxs = GROUP_IDX[g]
        out[b, idxs, :] = LAST_RESULTS.results[c]["out"][: len(idxs)]
    return out
